# revision 1
# baseline (speedup 1.0000x reference)
"""Trainium2 Bass kernel for nn_MLP_Route_RL_Model (route RL model).

Reference math (per batch element b of 256):
  - state = [route_nums (48) | customers (48*24*36)]
  - customer MLP (tanh-tanh, 36->128->32) on every node of every route
  - 2-layer GRU (hidden 128) over the 24 nodes of each of the 48 routes
  - route summary mean, node-selection MLP 256->256->128->24, masked softmax

Sharding: pure data parallel over batch B=256 -> 8 cores x 32.
Layout on device: feature-major activations ([feature, token] in SBUF) so
matmuls contract over the partition dim without transposes; the final logits
matmul uses the activations as the *stationary* operand to flip the output to
token-major for the free-dim softmax.
"""

import os
import sys

import numpy as np

sys.path.insert(0, "/opt/trn_rl_repo")

import concourse.bass as bass  # noqa: E402
import concourse.bacc as bacc  # noqa: E402
import concourse.mybir as mybir  # noqa: E402
import concourse.tile as tile  # noqa: E402
from concourse.bass_utils import run_bass_kernel_spmd  # noqa: E402

F32 = mybir.dt.float32
F32R = mybir.dt.float32r
F16 = mybir.dt.float16
AF = mybir.ActivationFunctionType
OP = mybir.AluOpType

# Problem shape constants
B = 256
NCORES = 8
BLOC = B // NCORES          # 32 batch rows per core
MR = 48                     # routes per batch
MN = 24                     # nodes per route
FEAT = 36
CH = 128                    # customer hidden
CO = 32                     # customer out
GH = 128                    # GRU hidden
S = BLOC * MR               # sequences per core = 1536
NC = 512                    # token chunk (PSUM bank width in fp32)
NCH = S // NC               # chunks per core = 3
NG = MN // 4                # node groups of 4 (cust_out partition stacking)

_cache = {}


def _build(reps=1):
    """Trace + schedule the per-core Tile kernel. Returns the Bass module.

    reps>1 repeats the whole computation (timing calibration only).
    """
    nc = bacc.Bacc("TRN2", target_bir_lowering=False, debug=False)

    # ---- DRAM I/O ----------------------------------------------------------
    d_cust = nc.dram_tensor("cust_fm", [FEAT, MN * S], F16, kind="ExternalInput")
    d_rn = nc.dram_tensor("rn_tm", [S, 1], F32, kind="ExternalInput")
    d_wc1 = nc.dram_tensor("Wc1h", [FEAT, CH], F16, kind="ExternalInput")
    d_bc1 = nc.dram_tensor("bc1", [CH, 1], F32, kind="ExternalInput")
    d_wc2 = nc.dram_tensor("Wc2h", [CH, CO], F16, kind="ExternalInput")
    d_bc2 = nc.dram_tensor("bc2s", [128, 1], F32, kind="ExternalInput")
    d_wih0 = nc.dram_tensor("Wih0h", [128, 3 * GH], F16, kind="ExternalInput")
    d_whh0 = nc.dram_tensor("Whh0h", [GH, 3 * GH], F16, kind="ExternalInput")
    d_wih1 = nc.dram_tensor("Wih1h", [GH, 3 * GH], F16, kind="ExternalInput")
    d_whh1 = nc.dram_tensor("Whh1h", [GH, 3 * GH], F16, kind="ExternalInput")
    d_gb = {}
    for layer in (0, 1):
        for g in ("r", "z", "in", "hn"):
            d_gb[(layer, g)] = nc.dram_tensor(
                f"b{layer}_{g}", [GH, 1], F32, kind="ExternalInput"
            )
    d_wn1a = nc.dram_tensor("Wn1a", [GH, 256], F16, kind="ExternalInput")
    d_wn1b = nc.dram_tensor("Wn1b", [GH, 256], F16, kind="ExternalInput")
    d_bn1 = nc.dram_tensor("bn1c", [128, 2], F32, kind="ExternalInput")
    d_wn2a = nc.dram_tensor("Wn2a", [128, 128], F16, kind="ExternalInput")
    d_wn2b = nc.dram_tensor("Wn2b", [128, 128], F16, kind="ExternalInput")
    d_bn2 = nc.dram_tensor("bn2c", [128, 1], F32, kind="ExternalInput")
    d_wn3 = nc.dram_tensor("Wn3h", [GH, MN], F16, kind="ExternalInput")
    d_bn3 = nc.dram_tensor("bn3r", [1, MN], F32, kind="ExternalInput")
    d_sel = nc.dram_tensor("sel", [BLOC, S], F16, kind="ExternalInput")
    d_iota = nc.dram_tensor("iota24", [128, MN], F32, kind="ExternalInput")
    d_ones = nc.dram_tensor("ones128", [1, 128], F32, kind="ExternalInput")
    d_out = nc.dram_tensor("out_tm", [S, MN], F32, kind="ExternalOutput")

    with tile.TileContext(nc) as tc:
        with (
            tc.tile_pool(name="wpool", bufs=1) as wp,
            tc.tile_pool(name="state", bufs=1) as sp,
            tc.tile_pool(name="xin", bufs=10) as xp,
            tc.tile_pool(name="h1c", bufs=10) as h1p,
            tc.tile_pool(name="gates", bufs=14) as gp,
            tc.tile_pool(name="upd", bufs=14) as up,
            tc.tile_pool(name="fin", bufs=4) as fp_,
            tc.tile_pool(name="ps", bufs=2, space="PSUM") as ps,
        ):
            # ---- load weights / constants (resident) -----------------------
            def wtile(dram, shape, dtype):
                t = wp.tile(shape, dtype, tag=dram.name)
                nc.sync.dma_start(t[:], dram.ap())
                return t

            wc1 = wtile(d_wc1, [FEAT, CH], F16)
            bc1 = wtile(d_bc1, [CH, 1], F32)
            wc2 = wtile(d_wc2, [CH, CO], F16)
            bc2 = wtile(d_bc2, [128, 1], F32)
            wih0 = wtile(d_wih0, [128, 3 * GH], F16)
            whh0 = wtile(d_whh0, [GH, 3 * GH], F16)
            wih1 = wtile(d_wih1, [GH, 3 * GH], F16)
            whh1 = wtile(d_whh1, [GH, 3 * GH], F16)
            gb = {}
            for k, d in d_gb.items():
                gb[k] = wtile(d, [GH, 1], F32)
            wn1a = wtile(d_wn1a, [GH, 256], F16)
            wn1b = wtile(d_wn1b, [GH, 256], F16)
            bn1 = wtile(d_bn1, [128, 2], F32)
            wn2a = wtile(d_wn2a, [128, 128], F16)
            wn2b = wtile(d_wn2b, [128, 128], F16)
            bn2 = wtile(d_bn2, [128, 1], F32)
            wn3 = wtile(d_wn3, [GH, MN], F16)
            bn3 = wtile(d_bn3, [1, MN], F32)
            sel = wtile(d_sel, [BLOC, S], F16)
            iota24 = wtile(d_iota, [128, MN], F32)
            ones128 = wtile(d_ones, [1, 128], F32)

            # persistent state: customer-MLP output, GRU hidden states
            # cust_out layout: partition = (n%4)*32 + f, free = (n//4)*S + s
            cust = sp.tile([128, NG * S], F16, tag="cust_out")
            h1 = sp.tile([GH, S], F16, tag="h1")
            h2 = sp.tile([GH, S], F16, tag="h2")

          # timing-calibration repeat loop (reps=1 in production)
          # fmt: off
            for _rep in range(reps):
              nc.vector.memset(h1[:], 0.0)
              nc.gpsimd.memset(h2[:], 0.0)

              # ---- phase A: customer MLP ----------------------------------
              # tokens ordered (node, seq); chunks of NC seqs
              xtiles = {}
              def emitA(g):
                  for sb in range(NCH):
                      gi = g * NCH + sb
                      c2 = ps.tile([128, NC], F32, tag="pb" if gi % 2 == 0 else "pd")
                      for k in range(4):
                          n = 4 * g + k
                          if n not in xtiles:
                              xn = xp.tile([FEAT, S], F16, tag="xc", name=f"xc{n}")
                              nc.sync.dma_start(
                                  xn[:], d_cust.ap()[:, n * S : (n + 1) * S]
                              )
                              xtiles[n] = xn
                          xc = xtiles[n]
                          p1 = ps.tile([CH, NC], F32, tag="pa" if k % 2 == 0 else "pc")
                          nc.tensor.matmul(p1[:], wc1[:], xc[:, sb * NC : (sb + 1) * NC])
                          h1c = h1p.tile([CH, NC], F16, tag="h1c")
                          nc.scalar.activation(h1c[:], p1[:], AF.Tanh, bias=bc1[:])
                          nc.tensor.matmul(
                              c2[32 * k : 32 * (k + 1), :], wc2[:], h1c[:],
                              tile_position=(0, 32 * k),
                          )
                      nc.scalar.activation(
                          cust[:, g * S + sb * NC : g * S + (sb + 1) * NC],
                          c2[:],
                          AF.Tanh,
                          bias=bc2[:],
                      )

              # ---- phase B: 2-layer GRU over MN steps -----------------------
              def gru_cell(xap, kq, wih, whh, layer, hfull, c0, c1):
                  """One GRU cell update on h[:, c0:c1] with input xap."""
                  h = hfull[:, c0:c1]
                  w = c1 - c0
                  pr = ps.tile([GH, NC], F32, tag="pa")
                  pz = ps.tile([GH, NC], F32, tag="pb")
                  pi = ps.tile([GH, NC], F32, tag="pc")
                  ph = ps.tile([GH, NC], F32, tag="pd")
                  if kq is not None:
                      p0 = 32 * kq
                      tp = (p0, 0)
                      wk = wih[p0 : p0 + CO, :]
                      nc.tensor.matmul(pr[:], whh[:, 0:GH], h, start=True, stop=False)
                      nc.tensor.matmul(pr[:], wk[:, 0:GH], xap, start=False, stop=True,
                                       tile_position=tp)
                      nc.tensor.matmul(ph[:], whh[:, 2 * GH : 3 * GH], h)
                      nc.tensor.matmul(pi[:], wk[:, 2 * GH : 3 * GH], xap, tile_position=tp)
                      nc.tensor.matmul(pz[:], whh[:, GH : 2 * GH], h, start=True, stop=False)
                      nc.tensor.matmul(pz[:], wk[:, GH : 2 * GH], xap, start=False, stop=True,
                                       tile_position=tp)
                  else:
                      nc.tensor.matmul(pr[:], whh[:, 0:GH], h, start=True, stop=False)
                      nc.tensor.matmul(pr[:], wih[:, 0:GH], xap, start=False, stop=True)
                      nc.tensor.matmul(ph[:], whh[:, 2 * GH : 3 * GH], h)
                      nc.tensor.matmul(pi[:], wih[:, 2 * GH : 3 * GH], xap)
                      nc.tensor.matmul(pz[:], whh[:, GH : 2 * GH], h, start=True, stop=False)
                      nc.tensor.matmul(pz[:], wih[:, GH : 2 * GH], xap, start=False, stop=True)
                  r = gp.tile([GH, w], F16, tag="r")
                  z = gp.tile([GH, w], F16, tag="z")
                  with tc.high_priority():
                      nc.scalar.activation(r[:], pr[:], AF.Sigmoid, bias=gb[(layer, "r")][:])
                  t_ = gp.tile([GH, w], F16, tag="t_")
                  with tc.high_priority():
                      nc.vector.scalar_tensor_tensor(
                          t_[:], ph[:], gb[(layer, "hn")][:], r[:], OP.add, OP.mult
                      )
                  s_ = gp.tile([GH, w], F16, tag="s_")
                  with tc.high_priority():
                      nc.vector.tensor_add(s_[:], pi[:], t_[:])
                  nc.scalar.activation(z[:], pz[:], AF.Sigmoid, bias=gb[(layer, "z")][:])
                  # u = z*h runs off the critical path (doesn't need n)
                  u_ = up.tile([GH, w], F16, tag="u_")
                  nc.gpsimd.tensor_mul(u_[:], z[:], h)
                  n_ = gp.tile([GH, w], F16, tag="n_")
                  # b_in folded into the tanh bias: tanh(s + b_in)
                  with tc.high_priority():
                      nc.scalar.activation(n_[:], s_[:], AF.Tanh, bias=gb[(layer, "in")][:])
                  # zm = z-1 (off-path, cheap 2x ts); v = zm*n ; h_new = u - v
                  zm = up.tile([GH, w], F16, tag="zm")
                  nc.vector.tensor_scalar(zm[:], z[:], 1.0, None, OP.subtract)
                  v_ = up.tile([GH, w], F16, tag="v_")
                  with tc.high_priority():
                      nc.vector.tensor_mul(v_[:], zm[:], n_[:])
                  if layer == 0:
                      with tc.high_priority():
                          nc.vector.tensor_sub(h, u_[:], v_[:])
                  else:
                      nc.gpsimd.tensor_sub(h, u_[:], v_[:])

              def emitB(t):
                  g, k = t // 4, t % 4
                  for c in range(NCH):
                      c0, c1 = c * NC, (c + 1) * NC
                      x0 = cust[32 * k : 32 * (k + 1), g * S + c0 : g * S + c1]
                      gru_cell(x0, k, wih0, whh0, 0, h1, c0, c1)
                      gru_cell(h1[:, c0:c1], None, wih1, whh1, 1, h2, c0, c1)

              # interleave: emit customer-MLP group g, then the 4 GRU steps
              # that consume it — lets the DVE-bound GRU overlap the
              # ACT/PE-bound customer MLP of later groups.
              for g in range(NG):
                  emitA(g)
                  for t in range(4 * g, 4 * g + 4):
                      emitB(t)

              # ---- phase C: route mean + node MLP + masked softmax ----------
              mean32 = fp_.tile([GH, BLOC], F32, tag="mean32")
              h2v = h2[:].rearrange("p (b r) -> p b r", r=MR)
              nc.vector.tensor_reduce(mean32[:], h2v, mybir.AxisListType.X, OP.add)
              mean = fp_.tile([GH, BLOC], F16, tag="mean")
              nc.vector.tensor_copy(mean[:], mean32[:])
              pmt = ps.tile([BLOC, 256], F32, tag="pc")
              nc.tensor.matmul(pmt[:], mean[:], wn1b[:])
              mmt = fp_.tile([BLOC, 256], F16, tag="mmt")
              nc.vector.tensor_copy(mmt[:], pmt[:])

              for c in range(NCH):
                  c0, c1 = c * NC, (c + 1) * NC
                  n1 = []
                  for m in range(2):
                      p1 = ps.tile([128, NC], F32, tag="pa")
                      nc.tensor.matmul(
                          p1[:], wn1a[:, 128 * m : 128 * (m + 1)], h2[:, c0:c1],
                          start=True, stop=False,
                      )
                      nc.tensor.matmul(
                          p1[:], mmt[:, 128 * m : 128 * (m + 1)], sel[:, c0:c1],
                          start=False, stop=True,
                      )
                      a1 = fp_.tile([128, NC], F16, tag=f"n1_{m}")
                      nc.scalar.activation(a1[:], p1[:], AF.Relu, bias=bn1[:, m : m + 1])
                      n1.append(a1)
                  p2 = ps.tile([128, NC], F32, tag="pb")
                  nc.tensor.matmul(p2[:], wn2a[:], n1[0][:], start=True, stop=False)
                  nc.tensor.matmul(p2[:], wn2b[:], n1[1][:], start=False, stop=True)
                  n2 = fp_.tile([128, NC], F16, tag="n2")
                  nc.scalar.activation(n2[:], p2[:], AF.Relu, bias=bn2[:])
                  for q in range(NC // 128):
                      tok0 = c0 + q * 128
                      pl = ps.tile([128, MN], F32, tag="pd")
                      nc.tensor.matmul(
                          pl[:], n2[:, q * 128 : (q + 1) * 128], wn3[:],
                          start=True, stop=False,
                      )
                      nc.tensor.matmul(pl[:], ones128[:], bn3[:], start=False, stop=True)
                      ex = fp_.tile([128, MN], F32, tag="ex")
                      sm = fp_.tile([128, 1], F32, tag="sm")
                      nc.scalar.activation(ex[:], pl[:], AF.Exp, accum_out=sm[:])
                      rec = fp_.tile([128, 1], F32, tag="rec")
                      nc.vector.reciprocal(rec[:], sm[:])
                      rnc = fp_.tile([128, 1], F32, tag="rnc")
                      nc.sync.dma_start(rnc[:], d_rn.ap()[tok0 : tok0 + 128, :])
                      msk = fp_.tile([128, MN], F32, tag="msk")
                      nc.vector.tensor_scalar(
                          msk[:], iota24[:], rnc[:], None, OP.is_lt
                      )
                      po = fp_.tile([128, MN], F32, tag="po")
                      nc.vector.scalar_tensor_tensor(
                          po[:], ex[:], rec[:], msk[:], OP.mult, OP.mult
                      )
                      nc.sync.dma_start(d_out.ap()[tok0 : tok0 + 128, :], po[:])

    nc.compile()
    return nc


def _prep_inputs(inputs):
    """Host-side preprocessing -> list of per-core input dicts."""
    state = np.ascontiguousarray(inputs["state"], dtype=np.float32)
    rn = state[:, :MR]                                    # [B, 48]
    cust = state[:, MR:].reshape(B, MR, MN, FEAT)

    def f32(x):
        return np.ascontiguousarray(np.asarray(x, dtype=np.float32))

    Wih0 = f32(inputs["Wih0"]); Whh0 = f32(inputs["Whh0"])
    Wih1 = f32(inputs["Wih1"]); Whh1 = f32(inputs["Whh1"])
    bih0 = f32(inputs["bih0"]); bhh0 = f32(inputs["bhh0"])
    bih1 = f32(inputs["bih1"]); bhh1 = f32(inputs["bhh1"])

    com = {
        "Wc1h": np.ascontiguousarray(np.asarray(inputs["Wc1"], np.float16)),
        "bc1": f32(inputs["bc1"]).reshape(CH, 1),
        "Wc2h": np.ascontiguousarray(np.asarray(inputs["Wc2"], np.float16)),
        "bc2s": np.tile(f32(inputs["bc2"]).reshape(CO), 4).reshape(128, 1),
        "Wih0h": np.ascontiguousarray(np.tile(np.asarray(Wih0, np.float16), (4, 1))),
        "Whh0h": Whh0.astype(np.float16), "Wih1h": Wih1.astype(np.float16),
        "Whh1h": Whh1.astype(np.float16),
        "b0_r": (bih0[0:GH] + bhh0[0:GH]).reshape(GH, 1),
        "b0_z": (bih0[GH : 2 * GH] + bhh0[GH : 2 * GH]).reshape(GH, 1),
        "b0_in": bih0[2 * GH :].reshape(GH, 1),
        "b0_hn": bhh0[2 * GH :].reshape(GH, 1),
        "b1_r": (bih1[0:GH] + bhh1[0:GH]).reshape(GH, 1),
        "b1_z": (bih1[GH : 2 * GH] + bhh1[GH : 2 * GH]).reshape(GH, 1),
        "b1_in": bih1[2 * GH :].reshape(GH, 1),
        "b1_hn": bhh1[2 * GH :].reshape(GH, 1),
        "Wn1a": f32(inputs["Wn1"])[0:GH, :].astype(np.float16),
        "Wn1b": (f32(inputs["Wn1"])[GH:, :] / np.float32(MR)).astype(np.float16),
        "bn1c": np.ascontiguousarray(f32(inputs["bn1"]).reshape(2, 128).T),
        "Wn2a": f32(inputs["Wn2"])[0:128, :].astype(np.float16),
        "Wn2b": f32(inputs["Wn2"])[128:256, :].astype(np.float16),
        "bn2c": f32(inputs["bn2"]).reshape(128, 1),
        "Wn3h": np.asarray(inputs["Wn3"], np.float16),
        "bn3r": f32(inputs["bn3"]).reshape(1, MN),
        "iota24": np.tile(np.arange(MN, dtype=np.float32), (128, 1)),
        "ones128": np.ones((1, 128), np.float32),
    }
    sel = np.zeros((BLOC, S), np.float32)
    sel[np.arange(S) // MR, np.arange(S)] = 1.0
    com["sel"] = sel.astype(np.float16)

    in_maps = []
    for core in range(NCORES):
        b0, b1 = core * BLOC, (core + 1) * BLOC
        # cust_fm[f, n*S + (b*MR+r)] = cust[b, r, n, f]
        cfm = cust[b0:b1].transpose(3, 2, 0, 1).reshape(FEAT, MN * S)
        m = dict(com)
        m["cust_fm"] = np.ascontiguousarray(cfm.astype(np.float16))
        m["rn_tm"] = np.ascontiguousarray(rn[b0:b1].reshape(S, 1))
        in_maps.append(m)
    return in_maps


def _run(inputs, **kw):
    if "nc" not in _cache:
        _cache["nc"] = _build()
    nc = _cache["nc"]
    in_maps = _prep_inputs(inputs)
    return run_bass_kernel_spmd(nc, in_maps, core_ids=list(range(NCORES)), **kw)


def kernel(**inputs) -> np.ndarray:
    res = _run(inputs)
    outs = [r["out_tm"] for r in res.results]
    return np.concatenate(outs, axis=0).reshape(B, MR, MN)



# revision 2
# speedup vs baseline: 9.8117x; 9.8117x over previous
"""Trainium2 Bass kernel for nn_MLP_Route_RL_Model (route RL model).

Reference math (per batch element b of 256):
  - state = [route_nums (48) | customers (48*24*36)]
  - customer MLP (tanh-tanh, 36->128->32) on every node of every route
  - 2-layer GRU (hidden 128) over the 24 nodes of each of the 48 routes
  - route summary mean, node-selection MLP 256->256->128->24, masked softmax

Sharding: pure data parallel over batch B=256 -> 8 cores x 32.

Layout: feature-major activations ([feature, token] in SBUF) so matmuls
contract over the partition dim without transposes.

v2 schedule notes (the kernel is ACT/DVE elementwise-bound):
  - GRU h' = n + z*(h-n): 3 ops (d_ on Pool, e_/h' on DVE at fp16 2x)
    instead of the 4-op (1-z)*n + z*h form.
  - n-gate tanh runs once per layer-step over the full 1536-token width
    (s_ staged in one SBUF tile) - fewer ACT fixed overheads.
  - PSUM: phase A gets 2 dedicated banks; GRU gates get pr/pz (1 buf) +
    ph/pi (2 bufs) = 6 banks, so gate matmuls never contend with the
    customer MLP and the pi-recycle serial cycle is halved.
  - Phase A is emitted at low scheduler priority: it fills ACT/PE gaps in
    the GRU steady state instead of preempting the recurrence.
"""

import os
import sys

import numpy as np

sys.path.insert(0, "/opt/trn_rl_repo")

import concourse.bass as bass  # noqa: E402
import concourse.bacc as bacc  # noqa: E402
import concourse.mybir as mybir  # noqa: E402
import concourse.tile as tile  # noqa: E402
from concourse.bass_utils import run_bass_kernel_spmd  # noqa: E402

F32 = mybir.dt.float32
F16 = mybir.dt.float16
AF = mybir.ActivationFunctionType
OP = mybir.AluOpType

# Problem shape constants
B = 256
NCORES = 8
BLOC = B // NCORES          # 32 batch rows per core
MR = 48                     # routes per batch
MN = 24                     # nodes per route
FEAT = 36
CH = 128                    # customer hidden
CO = 32                     # customer out
GH = 128                    # GRU hidden
S = BLOC * MR               # sequences per core = 1536
NC = 512                    # token chunk (PSUM bank width in fp32)
NCH = S // NC               # chunks per core = 3
NG = MN // 4                # node groups of 4 (cust_out partition stacking)

_cache = {}


def _build(reps=1):
    """Trace + schedule the per-core Tile kernel. Returns the Bass module."""
    nc = bacc.Bacc("TRN2", target_bir_lowering=False, debug=False)

    # ---- DRAM I/O ----------------------------------------------------------
    d_cust = nc.dram_tensor("cust_fm", [FEAT, MN * S], F16, kind="ExternalInput")
    d_rn = nc.dram_tensor("rn_tm", [S, 1], F32, kind="ExternalInput")
    d_wc1 = nc.dram_tensor("Wc1h", [FEAT, CH], F16, kind="ExternalInput")
    d_bc1 = nc.dram_tensor("bc1", [CH, 1], F32, kind="ExternalInput")
    d_wc2 = nc.dram_tensor("Wc2h", [CH, CO], F16, kind="ExternalInput")
    d_bc2 = nc.dram_tensor("bc2s", [128, 1], F32, kind="ExternalInput")
    d_wih0 = nc.dram_tensor("Wih0h", [128, 3 * GH], F16, kind="ExternalInput")
    d_whh0 = nc.dram_tensor("Whh0h", [GH, 3 * GH], F16, kind="ExternalInput")
    d_wih1 = nc.dram_tensor("Wih1h", [GH, 3 * GH], F16, kind="ExternalInput")
    d_whh1 = nc.dram_tensor("Whh1h", [GH, 3 * GH], F16, kind="ExternalInput")
    d_gb = {}
    for layer in (0, 1):
        for g in ("r", "z", "in", "hn"):
            d_gb[(layer, g)] = nc.dram_tensor(
                f"b{layer}_{g}", [GH, 1], F32, kind="ExternalInput"
            )
    d_wn1a = nc.dram_tensor("Wn1a", [GH, 256], F16, kind="ExternalInput")
    d_wn1b = nc.dram_tensor("Wn1b", [GH, 256], F16, kind="ExternalInput")
    d_bn1 = nc.dram_tensor("bn1c", [128, 2], F32, kind="ExternalInput")
    d_wn2a = nc.dram_tensor("Wn2a", [128, 128], F16, kind="ExternalInput")
    d_wn2b = nc.dram_tensor("Wn2b", [128, 128], F16, kind="ExternalInput")
    d_bn2 = nc.dram_tensor("bn2c", [128, 1], F32, kind="ExternalInput")
    d_wn3 = nc.dram_tensor("Wn3h", [GH, MN], F16, kind="ExternalInput")
    d_bn3 = nc.dram_tensor("bn3r", [1, MN], F32, kind="ExternalInput")
    d_sel = nc.dram_tensor("sel", [BLOC, S], F16, kind="ExternalInput")
    d_iota = nc.dram_tensor("iota24", [128, MN], F32, kind="ExternalInput")
    d_ones = nc.dram_tensor("ones128", [1, 128], F32, kind="ExternalInput")
    d_out = nc.dram_tensor("out_tm", [S, MN], F32, kind="ExternalOutput")

    with tile.TileContext(nc) as tc:
        with (
            tc.tile_pool(name="wpool", bufs=1) as wp,
            tc.tile_pool(name="state", bufs=1) as sp,
            tc.tile_pool(name="xin", bufs=10) as xp,
            tc.tile_pool(name="h1c", bufs=8) as h1p,
            tc.tile_pool(name="wk", bufs=6) as wk,
            tc.tile_pool(name="fw", bufs=2) as fw,
            tc.tile_pool(name="fin", bufs=4) as fp_,
            tc.tile_pool(name="psA", bufs=1, space="PSUM") as psA,
            tc.tile_pool(name="psRZ", bufs=1, space="PSUM") as psRZ,
            tc.tile_pool(name="psHI", bufs=2, space="PSUM") as psHI,
        ):
            # ---- load weights / constants (resident) -----------------------
            def wtile(dram, shape, dtype):
                t = wp.tile(shape, dtype, tag=dram.name)
                nc.sync.dma_start(t[:], dram.ap())
                return t

            wc1 = wtile(d_wc1, [FEAT, CH], F16)
            bc1 = wtile(d_bc1, [CH, 1], F32)
            wc2 = wtile(d_wc2, [CH, CO], F16)
            bc2 = wtile(d_bc2, [128, 1], F32)
            wih0 = wtile(d_wih0, [128, 3 * GH], F16)
            whh0 = wtile(d_whh0, [GH, 3 * GH], F16)
            wih1 = wtile(d_wih1, [GH, 3 * GH], F16)
            whh1 = wtile(d_whh1, [GH, 3 * GH], F16)
            gb = {}
            for k, d in d_gb.items():
                gb[k] = wtile(d, [GH, 1], F32)
            wn1a = wtile(d_wn1a, [GH, 256], F16)
            wn1b = wtile(d_wn1b, [GH, 256], F16)
            bn1 = wtile(d_bn1, [128, 2], F32)
            wn2a = wtile(d_wn2a, [128, 128], F16)
            wn2b = wtile(d_wn2b, [128, 128], F16)
            bn2 = wtile(d_bn2, [128, 1], F32)
            wn3 = wtile(d_wn3, [GH, MN], F16)
            bn3 = wtile(d_bn3, [1, MN], F32)
            sel = wtile(d_sel, [BLOC, S], F16)
            iota24 = wtile(d_iota, [128, MN], F32)
            ones128 = wtile(d_ones, [1, 128], F32)

            # persistent state: customer-MLP output, GRU hidden states
            # cust_out layout: partition = (n%4)*32 + f, free = (n//4)*S + s
            cust = sp.tile([128, NG * S], F16, tag="cust_out")
            h1 = sp.tile([GH, S], F16, tag="h1")
            h2 = sp.tile([GH, S], F16, tag="h2")

            def lowprio():
                # deprioritize: scheduler runs these only in recurrence gaps
                return tc.high_priority(offset=-1_000_000)

          # timing-calibration repeat loop (reps=1 in production)
          # fmt: off
            for _rep in range(reps):
              nc.vector.memset(h1[:], 0.0)
              nc.gpsimd.memset(h2[:], 0.0)

              # ---- phase A: customer MLP (gap filler, low priority) --------
              xtiles = {}
              def emitA(g):
                with lowprio():
                  for sb in range(NCH):
                      gi = g * NCH + sb
                      c2 = psA.tile([128, NC], F32, tag="pA2")
                      for k in range(4):
                          n = 4 * g + k
                          if n not in xtiles:
                              xn = xp.tile([FEAT, S], F16, tag="xc", name=f"xc{n}")
                              nc.sync.dma_start(
                                  xn[:], d_cust.ap()[:, n * S : (n + 1) * S]
                              )
                              xtiles[n] = xn
                          xc = xtiles[n]
                          p1 = psA.tile([CH, NC], F32, tag="pA1")
                          nc.tensor.matmul(p1[:], wc1[:], xc[:, sb * NC : (sb + 1) * NC])
                          h1c = h1p.tile([CH, NC], F16, tag="h1c")
                          nc.scalar.activation(h1c[:], p1[:], AF.Tanh, bias=bc1[:])
                          nc.tensor.matmul(
                              c2[32 * k : 32 * (k + 1), :], wc2[:], h1c[:],
                              tile_position=(0, 32 * k),
                          )
                      nc.scalar.activation(
                          cust[:, g * S + sb * NC : g * S + (sb + 1) * NC],
                          c2[:],
                          AF.Tanh,
                          bias=bc2[:],
                      )

              # ---- phase B: 2-layer GRU over MN steps -----------------------
              def emitB_layer(t, layer, h, wih, whh, kq):
                  """One GRU layer update for step t on hidden h [GH, S].

                  kq is the cust partition-quadrant (layer 0) or None
                  (layer 1, x = h1)."""
                  g = t // 4
                  s_ = fw.tile([GH, S], F16, tag=f"s{layer}")
                  z_ = fw.tile([GH, S], F16, tag=f"z{layer}")
                  for c in range(NCH):
                      c0, c1 = c * NC, (c + 1) * NC
                      hc = h[:, c0:c1]
                      pr = psRZ.tile([GH, NC], F32, tag="pr")
                      pz = psRZ.tile([GH, NC], F32, tag="pz")
                      ph = psHI.tile([GH, NC], F32, tag="ph")
                      pi = psHI.tile([GH, NC], F32, tag="pi")
                      if kq is not None:
                          p0 = 32 * kq
                          tp = (p0, 0)
                          xc = cust[p0 : p0 + CO, g * S + c0 : g * S + c1]
                          wx = wih[p0 : p0 + CO, :]
                          nc.tensor.matmul(pr[:], whh[:, 0:GH], hc, start=True, stop=False)
                          nc.tensor.matmul(pr[:], wx[:, 0:GH], xc, start=False, stop=True,
                                           tile_position=tp)
                          nc.tensor.matmul(pz[:], whh[:, GH : 2 * GH], hc, start=True, stop=False)
                          nc.tensor.matmul(pz[:], wx[:, GH : 2 * GH], xc, start=False, stop=True,
                                           tile_position=tp)
                          nc.tensor.matmul(ph[:], whh[:, 2 * GH : 3 * GH], hc)
                          nc.tensor.matmul(pi[:], wx[:, 2 * GH : 3 * GH], xc, tile_position=tp)
                      else:
                          xc = h1[:, c0:c1]
                          nc.tensor.matmul(pr[:], whh[:, 0:GH], hc, start=True, stop=False)
                          nc.tensor.matmul(pr[:], wih[:, 0:GH], xc, start=False, stop=True)
                          nc.tensor.matmul(pz[:], whh[:, GH : 2 * GH], hc, start=True, stop=False)
                          nc.tensor.matmul(pz[:], wih[:, GH : 2 * GH], xc, start=False, stop=True)
                          nc.tensor.matmul(ph[:], whh[:, 2 * GH : 3 * GH], hc)
                          nc.tensor.matmul(pi[:], wih[:, 2 * GH : 3 * GH], xc)
                      r_c = wk.tile([GH, NC], F16, tag="r")
                      nc.scalar.activation(r_c[:], pr[:], AF.Sigmoid, bias=gb[(layer, "r")][:])
                      nc.scalar.activation(z_[:, c0:c1], pz[:], AF.Sigmoid, bias=gb[(layer, "z")][:])
                      t_c = wk.tile([GH, NC], F16, tag="t_")
                      nc.vector.scalar_tensor_tensor(
                          t_c[:], ph[:], gb[(layer, "hn")][:], r_c[:], OP.add, OP.mult
                      )
                      nc.vector.tensor_add(s_[:, c0:c1], pi[:], t_c[:])
                  # n = tanh(s + b_in), full token width (one ACT op)
                  n_ = fw.tile([GH, S], F16, tag=f"n{layer}")
                  nc.scalar.activation(n_[:], s_[:], AF.Tanh, bias=gb[(layer, "in")][:])
                  # h' = n + z*(h - n), per chunk: d_ on Pool, e_/h' on DVE 2x
                  for c in range(NCH):
                      c0, c1 = c * NC, (c + 1) * NC
                      d_c = wk.tile([GH, NC], F16, tag="d_")
                      nc.gpsimd.tensor_sub(d_c[:], h[:, c0:c1], n_[:, c0:c1])
                      e_c = wk.tile([GH, NC], F16, tag="e_")
                      nc.vector.tensor_mul(e_c[:], z_[:, c0:c1], d_c[:])
                      nc.vector.tensor_add(h[:, c0:c1], n_[:, c0:c1], e_c[:])

              for t in range(MN):
                  if t % 4 == 0:
                      emitA(t // 4)
                  emitB_layer(t, 0, h1, wih0, whh0, t % 4)
                  emitB_layer(t, 1, h2, wih1, whh1, None)

              # ---- phase C: route mean + node MLP + masked softmax ----------
              mean32 = fp_.tile([GH, BLOC], F32, tag="mean32")
              h2v = h2[:].rearrange("p (b r) -> p b r", r=MR)
              nc.vector.tensor_reduce(mean32[:], h2v, mybir.AxisListType.X, OP.add)
              mean = fp_.tile([GH, BLOC], F16, tag="mean")
              nc.vector.tensor_copy(mean[:], mean32[:])
              pmt = psHI.tile([BLOC, 256], F32, tag="ph")
              nc.tensor.matmul(pmt[:], mean[:], wn1b[:])
              mmt = fp_.tile([BLOC, 256], F16, tag="mmt")
              nc.vector.tensor_copy(mmt[:], pmt[:])

              for c in range(NCH):
                  c0, c1 = c * NC, (c + 1) * NC
                  n1 = []
                  for m in range(2):
                      p1 = psHI.tile([128, NC], F32, tag="pi" if m == 0 else "ph")
                      nc.tensor.matmul(
                          p1[:], wn1a[:, 128 * m : 128 * (m + 1)], h2[:, c0:c1],
                          start=True, stop=False,
                      )
                      nc.tensor.matmul(
                          p1[:], mmt[:, 128 * m : 128 * (m + 1)], sel[:, c0:c1],
                          start=False, stop=True,
                      )
                      a1 = fp_.tile([128, NC], F16, tag=f"n1_{m}")
                      nc.scalar.activation(a1[:], p1[:], AF.Relu, bias=bn1[:, m : m + 1])
                      n1.append(a1)
                  p2 = psRZ.tile([128, NC], F32, tag="pr")
                  nc.tensor.matmul(p2[:], wn2a[:], n1[0][:], start=True, stop=False)
                  nc.tensor.matmul(p2[:], wn2b[:], n1[1][:], start=False, stop=True)
                  n2 = fp_.tile([128, NC], F16, tag="n2")
                  nc.scalar.activation(n2[:], p2[:], AF.Relu, bias=bn2[:])
                  for q in range(NC // 128):
                      tok0 = c0 + q * 128
                      pl = psRZ.tile([128, MN], F32, tag="pz")
                      nc.tensor.matmul(
                          pl[:], n2[:, q * 128 : (q + 1) * 128], wn3[:],
                          start=True, stop=False,
                      )
                      nc.tensor.matmul(pl[:], ones128[:], bn3[:], start=False, stop=True)
                      ex = fp_.tile([128, MN], F32, tag="ex")
                      sm = fp_.tile([128, 1], F32, tag="sm")
                      nc.scalar.activation(ex[:], pl[:], AF.Exp, accum_out=sm[:])
                      rec = fp_.tile([128, 1], F32, tag="rec")
                      nc.vector.reciprocal(rec[:], sm[:])
                      rnc = fp_.tile([128, 1], F32, tag="rnc")
                      nc.sync.dma_start(rnc[:], d_rn.ap()[tok0 : tok0 + 128, :])
                      msk = fp_.tile([128, MN], F32, tag="msk")
                      nc.vector.tensor_scalar(
                          msk[:], iota24[:], rnc[:], None, OP.is_lt
                      )
                      po = fp_.tile([128, MN], F32, tag="po")
                      nc.vector.scalar_tensor_tensor(
                          po[:], ex[:], rec[:], msk[:], OP.mult, OP.mult
                      )
                      nc.sync.dma_start(d_out.ap()[tok0 : tok0 + 128, :], po[:])

    nc.compile()
    return nc


def _prep_inputs(inputs):
    """Host-side preprocessing -> list of per-core input dicts."""
    state = np.ascontiguousarray(inputs["state"], dtype=np.float32)
    rn = state[:, :MR]                                    # [B, 48]
    cust = state[:, MR:].reshape(B, MR, MN, FEAT)

    def f32(x):
        return np.ascontiguousarray(np.asarray(x, dtype=np.float32))

    Wih0 = f32(inputs["Wih0"]); Whh0 = f32(inputs["Whh0"])
    Wih1 = f32(inputs["Wih1"]); Whh1 = f32(inputs["Whh1"])
    bih0 = f32(inputs["bih0"]); bhh0 = f32(inputs["bhh0"])
    bih1 = f32(inputs["bih1"]); bhh1 = f32(inputs["bhh1"])

    com = {
        "Wc1h": np.ascontiguousarray(np.asarray(inputs["Wc1"], np.float16)),
        "bc1": f32(inputs["bc1"]).reshape(CH, 1),
        "Wc2h": np.ascontiguousarray(np.asarray(inputs["Wc2"], np.float16)),
        "bc2s": np.tile(f32(inputs["bc2"]).reshape(CO), 4).reshape(128, 1),
        "Wih0h": np.ascontiguousarray(np.tile(np.asarray(Wih0, np.float16), (4, 1))),
        "Whh0h": Whh0.astype(np.float16), "Wih1h": Wih1.astype(np.float16),
        "Whh1h": Whh1.astype(np.float16),
        "b0_r": (bih0[0:GH] + bhh0[0:GH]).reshape(GH, 1),
        "b0_z": (bih0[GH : 2 * GH] + bhh0[GH : 2 * GH]).reshape(GH, 1),
        "b0_in": bih0[2 * GH :].reshape(GH, 1),
        "b0_hn": bhh0[2 * GH :].reshape(GH, 1),
        "b1_r": (bih1[0:GH] + bhh1[0:GH]).reshape(GH, 1),
        "b1_z": (bih1[GH : 2 * GH] + bhh1[GH : 2 * GH]).reshape(GH, 1),
        "b1_in": bih1[2 * GH :].reshape(GH, 1),
        "b1_hn": bhh1[2 * GH :].reshape(GH, 1),
        "Wn1a": f32(inputs["Wn1"])[0:GH, :].astype(np.float16),
        "Wn1b": (f32(inputs["Wn1"])[GH:, :] / np.float32(MR)).astype(np.float16),
        "bn1c": np.ascontiguousarray(f32(inputs["bn1"]).reshape(2, 128).T),
        "Wn2a": f32(inputs["Wn2"])[0:128, :].astype(np.float16),
        "Wn2b": f32(inputs["Wn2"])[128:256, :].astype(np.float16),
        "bn2c": f32(inputs["bn2"]).reshape(128, 1),
        "Wn3h": np.asarray(inputs["Wn3"], np.float16),
        "bn3r": f32(inputs["bn3"]).reshape(1, MN),
        "iota24": np.tile(np.arange(MN, dtype=np.float32), (128, 1)),
        "ones128": np.ones((1, 128), np.float32),
    }
    sel = np.zeros((BLOC, S), np.float32)
    sel[np.arange(S) // MR, np.arange(S)] = 1.0
    com["sel"] = sel.astype(np.float16)

    in_maps = []
    for core in range(NCORES):
        b0, b1 = core * BLOC, (core + 1) * BLOC
        # cust_fm[f, n*S + (b*MR+r)] = cust[b, r, n, f]
        cfm = cust[b0:b1].transpose(3, 2, 0, 1).reshape(FEAT, MN * S)
        m = dict(com)
        m["cust_fm"] = np.ascontiguousarray(cfm.astype(np.float16))
        m["rn_tm"] = np.ascontiguousarray(rn[b0:b1].reshape(S, 1))
        in_maps.append(m)
    return in_maps


def _run(inputs, **kw):
    if "nc" not in _cache:
        _cache["nc"] = _build()
    nc = _cache["nc"]
    in_maps = _prep_inputs(inputs)
    return run_bass_kernel_spmd(nc, in_maps, core_ids=list(range(NCORES)), **kw)


def kernel(**inputs) -> np.ndarray:
    res = _run(inputs)
    outs = [r["out_tm"] for r in res.results]
    return np.concatenate(outs, axis=0).reshape(B, MR, MN)


# revision 12
# speedup vs baseline: 14.2682x; 1.4542x over previous
"""Trainium2 Bass kernel for nn_MLP_Route_RL_Model (route RL model).

Reference math (per batch element b of 256):
  - state = [route_nums (48) | customers (48*24*36)]
  - customer MLP (tanh-tanh, 36->128->32) on every node of every route
  - 2-layer GRU (hidden 128) over the 24 nodes of each of the 48 routes
  - route summary mean, node-selection MLP 256->256->128->24, masked softmax

Sharding: pure data parallel over batch B=256 -> 8 cores x 32.

Layout: feature-major activations ([feature, token] in SBUF) so matmuls
contract over the partition dim without transposes.

v3 schedule notes (the kernel is ACT elementwise-bound):
  - GRU h' = n + z*(h-n): 3 DVE ops at fp16 2x instead of the 4-op
    (1-z)*n + z*h form.
  - n-gate input (pi + r*(ph+bhn)) is finished inside PSUM: after the
    DVE stt computes t_ = (ph+bhn)*r, an identity matmul accumulates
    I @ t_ into the pi bank (PE is idle-rich), so tanh reads the bank
    directly and the separate s_ = pi + t_ DVE add disappears.
  - All four gate PSUM tags have 2 dynamic slots (8 banks total); the
    customer MLP borrows ph/pi slots at low scheduler priority, filling
    ACT/PE gaps in the GRU steady state instead of preempting it.
"""

import os
import sys

import numpy as np

sys.path.insert(0, "/opt/trn_rl_repo")

import concourse.bass as bass  # noqa: E402
import concourse.bacc as bacc  # noqa: E402
import concourse.mybir as mybir  # noqa: E402
import concourse.tile as tile  # noqa: E402
from concourse.bass_utils import run_bass_kernel_spmd  # noqa: E402

F32 = mybir.dt.float32
F16 = mybir.dt.float16
AF = mybir.ActivationFunctionType
OP = mybir.AluOpType

# Problem shape constants
B = 256
NCORES = 8
BLOC = B // NCORES          # 32 batch rows per core
MR = 48                     # routes per batch
MN = 24                     # nodes per route
FEAT = 36
CH = 128                    # customer hidden
CO = 32                     # customer out
GH = 128                    # GRU hidden
S = BLOC * MR               # sequences per core = 1536
NC = 512                    # token chunk (PSUM bank width in fp32)
NCH = S // NC               # chunks per core = 3
NG = MN // 4                # node groups of 4 (cust_out partition stacking)

_cache = {}


def _build(reps=1):
    """Trace + schedule the per-core Tile kernel. Returns the Bass module."""
    nc = bacc.Bacc("TRN2", target_bir_lowering=False, debug=False)

    # ---- DRAM I/O ----------------------------------------------------------
    d_cust = nc.dram_tensor("cust_fm", [FEAT, MN * S], F16, kind="ExternalInput")
    d_rn = nc.dram_tensor("rn_tm", [S, 1], F32, kind="ExternalInput")
    d_wc1 = nc.dram_tensor("Wc1h", [FEAT, CH], F16, kind="ExternalInput")
    d_bc1 = nc.dram_tensor("bc1", [CH, 1], F32, kind="ExternalInput")
    d_wc2 = nc.dram_tensor("Wc2h", [CH, CO], F16, kind="ExternalInput")
    d_bc2 = nc.dram_tensor("bc2s", [128, 1], F32, kind="ExternalInput")
    d_wih0 = nc.dram_tensor("Wih0h", [128, 3 * GH], F16, kind="ExternalInput")
    d_whh0 = nc.dram_tensor("Whh0h", [GH, 3 * GH], F16, kind="ExternalInput")
    d_wih1 = nc.dram_tensor("Wih1h", [GH, 3 * GH], F16, kind="ExternalInput")
    d_whh1 = nc.dram_tensor("Whh1h", [GH, 3 * GH], F16, kind="ExternalInput")
    d_gb = {}
    for layer in (0, 1):
        for g in ("r", "z", "in", "hn"):
            d_gb[(layer, g)] = nc.dram_tensor(
                f"b{layer}_{g}", [GH, 1], F32, kind="ExternalInput"
            )
    d_wn1a = nc.dram_tensor("Wn1a", [GH, 256], F16, kind="ExternalInput")
    d_wn1b = nc.dram_tensor("Wn1b", [GH, 256], F16, kind="ExternalInput")
    d_bn1 = nc.dram_tensor("bn1c", [128, 2], F32, kind="ExternalInput")
    d_wn2a = nc.dram_tensor("Wn2a", [128, 128], F16, kind="ExternalInput")
    d_wn2b = nc.dram_tensor("Wn2b", [128, 128], F16, kind="ExternalInput")
    d_bn2 = nc.dram_tensor("bn2c", [128, 1], F32, kind="ExternalInput")
    d_wn3 = nc.dram_tensor("Wn3h", [GH, MN], F16, kind="ExternalInput")
    d_bn3 = nc.dram_tensor("bn3r", [1, MN], F32, kind="ExternalInput")
    d_sel = nc.dram_tensor("sel", [BLOC, S], F16, kind="ExternalInput")
    d_iota = nc.dram_tensor("iota24", [128, MN], F32, kind="ExternalInput")
    d_ones = nc.dram_tensor("ones128", [1, 128], F32, kind="ExternalInput")
    d_ident = nc.dram_tensor("ident128", [128, 128], F16, kind="ExternalInput")
    d_out = nc.dram_tensor("out_tm", [S, MN], F32, kind="ExternalOutput")

    with tile.TileContext(nc) as tc:
        with (
            tc.tile_pool(name="wpool", bufs=1) as wp,
            tc.tile_pool(name="state", bufs=1) as sp,
            tc.tile_pool(name="xin", bufs=10) as xp,
            tc.tile_pool(name="h1c", bufs=8) as h1p,
            tc.tile_pool(name="wk", bufs=6) as wk,
            tc.tile_pool(name="fin", bufs=4) as fp_,
            tc.tile_pool(name="ps", bufs=2, space="PSUM") as ps,
        ):
            # ---- load weights / constants (resident) -----------------------
            def wtile(dram, shape, dtype):
                t = wp.tile(shape, dtype, tag=dram.name)
                nc.sync.dma_start(t[:], dram.ap())
                return t

            wc1 = wtile(d_wc1, [FEAT, CH], F16)
            bc1 = wtile(d_bc1, [CH, 1], F32)
            wc2 = wtile(d_wc2, [CH, CO], F16)
            bc2 = wtile(d_bc2, [128, 1], F32)
            wih0 = wtile(d_wih0, [128, 3 * GH], F16)
            whh0 = wtile(d_whh0, [GH, 3 * GH], F16)
            wih1 = wtile(d_wih1, [GH, 3 * GH], F16)
            whh1 = wtile(d_whh1, [GH, 3 * GH], F16)
            gb = {}
            for k, d in d_gb.items():
                gb[k] = wtile(d, [GH, 1], F32)
            wn1a = wtile(d_wn1a, [GH, 256], F16)
            wn1b = wtile(d_wn1b, [GH, 256], F16)
            bn1 = wtile(d_bn1, [128, 2], F32)
            wn2a = wtile(d_wn2a, [128, 128], F16)
            wn2b = wtile(d_wn2b, [128, 128], F16)
            bn2 = wtile(d_bn2, [128, 1], F32)
            wn3 = wtile(d_wn3, [GH, MN], F16)
            bn3 = wtile(d_bn3, [1, MN], F32)
            sel = wtile(d_sel, [BLOC, S], F16)
            iota24 = wtile(d_iota, [128, MN], F32)
            ones128 = wtile(d_ones, [1, 128], F32)
            ident = wtile(d_ident, [128, 128], F16)

            # persistent state: customer-MLP output, GRU hidden states
            # cust_out layout: partition = (n%4)*32 + f, free = (n//4)*S + s
            cust = sp.tile([128, NG * S], F16, tag="cust_out")
            h1 = sp.tile([GH, S], F16, tag="h1")
            h2 = sp.tile([GH, S], F16, tag="h2")

            def lowprio():
                # deprioritize: scheduler runs these only in recurrence gaps
                return tc.high_priority(offset=-1_000_000)

          # timing-calibration repeat loop (reps=1 in production)
          # fmt: off
            for _rep in range(reps):
              nc.vector.memset(h1[:], 0.0)
              nc.gpsimd.memset(h2[:], 0.0)

              # ---- phase A: customer MLP (gap filler, low priority) --------
              xtiles = {}
              def emitA(g):
                with lowprio():
                  for sb in range(NCH):
                      c2 = ps.tile([128, NC], F32, tag="pi")
                      for k in range(4):
                          n = 4 * g + k
                          if n not in xtiles:
                              xn = xp.tile([FEAT, S], F16, tag="xc", name=f"xc{n}")
                              nc.sync.dma_start(
                                  xn[:], d_cust.ap()[:, n * S : (n + 1) * S]
                              )
                              xtiles[n] = xn
                          xc = xtiles[n]
                          p1 = ps.tile([CH, NC], F32, tag="ph")
                          nc.tensor.matmul(p1[:], wc1[:], xc[:, sb * NC : (sb + 1) * NC])
                          h1c = h1p.tile([CH, NC], F16, tag="h1c")
                          nc.scalar.activation(h1c[:], p1[:], AF.Tanh, bias=bc1[:])
                          nc.tensor.matmul(
                              c2[32 * k : 32 * (k + 1), :], wc2[:], h1c[:],
                              tile_position=(0, 32 * k),
                          )
                      nc.scalar.activation(
                          cust[:, g * S + sb * NC : g * S + (sb + 1) * NC],
                          c2[:],
                          AF.Tanh,
                          bias=bc2[:],
                      )

              # ---- phase B: 2-layer GRU over MN steps -----------------------
              def emitB_layer(t, layer, h, wih, whh, kq):
                  """One GRU layer update for step t on hidden h [GH, S].

                  kq is the cust partition-quadrant (layer 0) or None
                  (layer 1, x = h1). The three 512-token chunks are three
                  independent recurrences."""
                  g = t // 4
                  for c in range(NCH):
                      c0, c1 = c * NC, (c + 1) * NC
                      hc = h[:, c0:c1]
                      pr = ps.tile([GH, NC], F32, tag="pr")
                      pz = ps.tile([GH, NC], F32, tag="pz")
                      ph = ps.tile([GH, NC], F32, tag="ph")
                      pi = ps.tile([GH, NC], F32, tag="pi")
                      if kq is not None:
                          p0 = 32 * kq
                          tp = (p0, 0)
                          xc = cust[p0 : p0 + CO, g * S + c0 : g * S + c1]
                          wx = wih[p0 : p0 + CO, :]
                          nc.tensor.matmul(pr[:], whh[:, 0:GH], hc, start=True, stop=False)
                          nc.tensor.matmul(pr[:], wx[:, 0:GH], xc, start=False, stop=True,
                                           tile_position=tp)
                          nc.tensor.matmul(pz[:], whh[:, GH : 2 * GH], hc, start=True, stop=False)
                          nc.tensor.matmul(pz[:], wx[:, GH : 2 * GH], xc, start=False, stop=True,
                                           tile_position=tp)
                          nc.tensor.matmul(ph[:], whh[:, 2 * GH : 3 * GH], hc)
                      else:
                          xc = h1[:, c0:c1]
                          nc.tensor.matmul(pr[:], whh[:, 0:GH], hc, start=True, stop=False)
                          nc.tensor.matmul(pr[:], wih[:, 0:GH], xc, start=False, stop=True)
                          nc.tensor.matmul(pz[:], whh[:, GH : 2 * GH], hc, start=True, stop=False)
                          nc.tensor.matmul(pz[:], wih[:, GH : 2 * GH], xc, start=False, stop=True)
                          nc.tensor.matmul(ph[:], whh[:, 2 * GH : 3 * GH], hc)
                      r_c = wk.tile([GH, NC], F16, tag="r")
                      nc.scalar.activation(r_c[:], pr[:], AF.Sigmoid, bias=gb[(layer, "r")][:])
                      z_c = wk.tile([GH, NC], F16, tag="z")
                      nc.scalar.activation(z_c[:], pz[:], AF.Sigmoid, bias=gb[(layer, "z")][:])
                      t_c = wk.tile([GH, NC], F16, tag="t_")
                      nc.vector.scalar_tensor_tensor(
                          t_c[:], ph[:], gb[(layer, "hn")][:], r_c[:], OP.add, OP.mult
                      )
                      # finish the n-gate input inside PSUM: pi = I@t_ + Wih_n@x
                      nc.tensor.matmul(pi[:], ident[:], t_c[:], start=True, stop=False)
                      if kq is not None:
                          nc.tensor.matmul(pi[:], wx[:, 2 * GH : 3 * GH], xc,
                                           start=False, stop=True, tile_position=tp)
                      else:
                          nc.tensor.matmul(pi[:], wih[:, 2 * GH : 3 * GH], xc,
                                           start=False, stop=True)
                      n_c = wk.tile([GH, NC], F16, tag="n")
                      nc.scalar.activation(n_c[:], pi[:], AF.Tanh, bias=gb[(layer, "in")][:])
                      # h' = n + z*(h - n), all DVE at fp16 2x
                      d_c = wk.tile([GH, NC], F16, tag="d_")
                      nc.vector.tensor_sub(d_c[:], hc, n_c[:])
                      e_c = wk.tile([GH, NC], F16, tag="e_")
                      nc.vector.tensor_mul(e_c[:], z_c[:], d_c[:])
                      nc.vector.tensor_add(hc, n_c[:], e_c[:])

              for t in range(MN):
                  if t % 4 == 0:
                      emitA(t // 4)
                  emitB_layer(t, 0, h1, wih0, whh0, t % 4)
                  emitB_layer(t, 1, h2, wih1, whh1, None)

              # ---- phase C: route mean + node MLP + masked softmax ----------
              mean32 = fp_.tile([GH, BLOC], F32, tag="mean32")
              h2v = h2[:].rearrange("p (b r) -> p b r", r=MR)
              nc.vector.tensor_reduce(mean32[:], h2v, mybir.AxisListType.X, OP.add)
              mean = fp_.tile([GH, BLOC], F16, tag="mean")
              nc.vector.tensor_copy(mean[:], mean32[:])
              pmt = ps.tile([BLOC, 256], F32, tag="ph")
              nc.tensor.matmul(pmt[:], mean[:], wn1b[:])
              mmt = fp_.tile([BLOC, 256], F16, tag="mmt")
              nc.vector.tensor_copy(mmt[:], pmt[:])

              for c in range(NCH):
                  c0, c1 = c * NC, (c + 1) * NC
                  n1 = []
                  for m in range(2):
                      p1 = ps.tile([128, NC], F32, tag="pi" if m == 0 else "ph")
                      nc.tensor.matmul(
                          p1[:], wn1a[:, 128 * m : 128 * (m + 1)], h2[:, c0:c1],
                          start=True, stop=False,
                      )
                      nc.tensor.matmul(
                          p1[:], mmt[:, 128 * m : 128 * (m + 1)], sel[:, c0:c1],
                          start=False, stop=True,
                      )
                      a1 = fp_.tile([128, NC], F16, tag=f"n1_{m}")
                      nc.scalar.activation(a1[:], p1[:], AF.Relu, bias=bn1[:, m : m + 1])
                      n1.append(a1)
                  p2 = ps.tile([128, NC], F32, tag="pr")
                  nc.tensor.matmul(p2[:], wn2a[:], n1[0][:], start=True, stop=False)
                  nc.tensor.matmul(p2[:], wn2b[:], n1[1][:], start=False, stop=True)
                  n2 = fp_.tile([128, NC], F16, tag="n2")
                  nc.scalar.activation(n2[:], p2[:], AF.Relu, bias=bn2[:])
                  for q in range(NC // 128):
                      tok0 = c0 + q * 128
                      pl = ps.tile([128, MN], F32, tag="pz")
                      nc.tensor.matmul(
                          pl[:], n2[:, q * 128 : (q + 1) * 128], wn3[:],
                          start=True, stop=False,
                      )
                      nc.tensor.matmul(pl[:], ones128[:], bn3[:], start=False, stop=True)
                      ex = fp_.tile([128, MN], F32, tag="ex")
                      sm = fp_.tile([128, 1], F32, tag="sm")
                      nc.scalar.activation(ex[:], pl[:], AF.Exp, accum_out=sm[:])
                      rec = fp_.tile([128, 1], F32, tag="rec")
                      nc.vector.reciprocal(rec[:], sm[:])
                      rnc = fp_.tile([128, 1], F32, tag="rnc")
                      nc.sync.dma_start(rnc[:], d_rn.ap()[tok0 : tok0 + 128, :])
                      msk = fp_.tile([128, MN], F32, tag="msk")
                      nc.vector.tensor_scalar(
                          msk[:], iota24[:], rnc[:], None, OP.is_lt
                      )
                      po = fp_.tile([128, MN], F32, tag="po")
                      nc.vector.scalar_tensor_tensor(
                          po[:], ex[:], rec[:], msk[:], OP.mult, OP.mult
                      )
                      nc.sync.dma_start(d_out.ap()[tok0 : tok0 + 128, :], po[:])

    nc.compile()
    return nc


def _prep_inputs(inputs):
    """Host-side preprocessing -> list of per-core input dicts."""
    state = np.ascontiguousarray(inputs["state"], dtype=np.float32)
    rn = state[:, :MR]                                    # [B, 48]
    cust = state[:, MR:].reshape(B, MR, MN, FEAT)

    def f32(x):
        return np.ascontiguousarray(np.asarray(x, dtype=np.float32))

    Wih0 = f32(inputs["Wih0"]); Whh0 = f32(inputs["Whh0"])
    Wih1 = f32(inputs["Wih1"]); Whh1 = f32(inputs["Whh1"])
    bih0 = f32(inputs["bih0"]); bhh0 = f32(inputs["bhh0"])
    bih1 = f32(inputs["bih1"]); bhh1 = f32(inputs["bhh1"])

    com = {
        "Wc1h": np.ascontiguousarray(np.asarray(inputs["Wc1"], np.float16)),
        "bc1": f32(inputs["bc1"]).reshape(CH, 1),
        "Wc2h": np.ascontiguousarray(np.asarray(inputs["Wc2"], np.float16)),
        "bc2s": np.tile(f32(inputs["bc2"]).reshape(CO), 4).reshape(128, 1),
        "Wih0h": np.ascontiguousarray(np.tile(np.asarray(Wih0, np.float16), (4, 1))),
        "Whh0h": Whh0.astype(np.float16), "Wih1h": Wih1.astype(np.float16),
        "Whh1h": Whh1.astype(np.float16),
        "b0_r": (bih0[0:GH] + bhh0[0:GH]).reshape(GH, 1),
        "b0_z": (bih0[GH : 2 * GH] + bhh0[GH : 2 * GH]).reshape(GH, 1),
        "b0_in": bih0[2 * GH :].reshape(GH, 1),
        "b0_hn": bhh0[2 * GH :].reshape(GH, 1),
        "b1_r": (bih1[0:GH] + bhh1[0:GH]).reshape(GH, 1),
        "b1_z": (bih1[GH : 2 * GH] + bhh1[GH : 2 * GH]).reshape(GH, 1),
        "b1_in": bih1[2 * GH :].reshape(GH, 1),
        "b1_hn": bhh1[2 * GH :].reshape(GH, 1),
        "Wn1a": f32(inputs["Wn1"])[0:GH, :].astype(np.float16),
        "Wn1b": (f32(inputs["Wn1"])[GH:, :] / np.float32(MR)).astype(np.float16),
        "bn1c": np.ascontiguousarray(f32(inputs["bn1"]).reshape(2, 128).T),
        "Wn2a": f32(inputs["Wn2"])[0:128, :].astype(np.float16),
        "Wn2b": f32(inputs["Wn2"])[128:256, :].astype(np.float16),
        "bn2c": f32(inputs["bn2"]).reshape(128, 1),
        "Wn3h": np.asarray(inputs["Wn3"], np.float16),
        "bn3r": f32(inputs["bn3"]).reshape(1, MN),
        "iota24": np.tile(np.arange(MN, dtype=np.float32), (128, 1)),
        "ones128": np.ones((1, 128), np.float32),
        "ident128": np.eye(128, dtype=np.float16),
    }
    sel = np.zeros((BLOC, S), np.float32)
    sel[np.arange(S) // MR, np.arange(S)] = 1.0
    com["sel"] = sel.astype(np.float16)

    in_maps = []
    for core in range(NCORES):
        b0, b1 = core * BLOC, (core + 1) * BLOC
        # cust_fm[f, n*S + (b*MR+r)] = cust[b, r, n, f]
        cfm = cust[b0:b1].transpose(3, 2, 0, 1).reshape(FEAT, MN * S)
        m = dict(com)
        m["cust_fm"] = np.ascontiguousarray(cfm.astype(np.float16))
        m["rn_tm"] = np.ascontiguousarray(rn[b0:b1].reshape(S, 1))
        in_maps.append(m)
    return in_maps


def _run(inputs, **kw):
    if "nc" not in _cache:
        _cache["nc"] = _build()
    nc = _cache["nc"]
    in_maps = _prep_inputs(inputs)
    return run_bass_kernel_spmd(nc, in_maps, core_ids=list(range(NCORES)), **kw)


def kernel(**inputs) -> np.ndarray:
    res = _run(inputs)
    outs = [r["out_tm"] for r in res.results]
    return np.concatenate(outs, axis=0).reshape(B, MR, MN)
